# revision 16
# baseline (speedup 1.0000x reference)
"""Trainium2 Bass kernel for fused attention block (QKV proj + per-head RMSNorm
+ RoPE + causal attention + o_proj), sharded over 8 NeuronCores as
2-way batch-parallel x 4-way head-parallel (tensor parallel).

Each core computes, for one batch element b and 4 heads:
    qkv = x[b] @ w_shard.T          (bf16 matmul, fp32 accum)
    per-head RMSNorm, RoPE(q, k)
    S^T = K Q^T / sqrt(HD); P^T = exp(S^T) * causal_mask   (no max needed:
        RMS-normalized q,k bound |score| <= sqrt(HD) ~ 11.3)
    O^T = V^T P^T; denom = ones^T P^T; O^T /= denom
    y_partial = O @ w_o_cols.T      -> host sums the 4 head-group partials.
"""

import numpy as np
import ml_dtypes

import concourse.bass as bass
import concourse.mybir as mybir
import concourse.tile as tile
from concourse.bass_utils import run_bass_kernel_spmd

BF16 = mybir.dt.bfloat16
F32 = mybir.dt.float32
AF = mybir.ActivationFunctionType
OP = mybir.AluOpType

B, S, DM = 2, 2048, 2048
HD, H = 128, 16
NH = 4              # heads per core
EPS = 1e-6
SCALE = 1.0 / np.sqrt(HD)
NTC = S // 128      # 16 token chunks
NDM = DM // 128     # 16 dm chunks
NQC = S // 512      # 4 q chunks of 512


def legalize_multi_waits(nc):
    """Walrus codegen only supports one sync wait per instruction; move
    extras onto preceding no-op carriers on the same engine."""
    n = 0
    for f in nc.m.functions:
        for blk in f.blocks:
            newlist = []
            for ins in blk.instructions:
                si = ins.sync_info
                if si is not None and len(si.on_wait) > 1:
                    waits = list(si.on_wait)
                    for i, w in enumerate(waits[:-1]):
                        nop = mybir.InstNoOp(
                            name=f"{ins.name}-wsplit{i}",
                            sync_info=mybir.SyncInfo(on_wait=[w], on_update=[]),
                            bass_nofuse=True,
                            engine=ins.engine,
                        )
                        newlist.append(nop)
                        n += 1
                    ins.sync_info = mybir.SyncInfo(
                        on_wait=[waits[-1]], on_update=list(si.on_update)
                    )
                newlist.append(ins)
            blk.instructions = newlist
    return n


def build_nc(repeat=1):
    nc = bass.Bass()
    xT = nc.declare_dram_parameter("xT", [DM, S], BF16, isOutput=False)
    wq = nc.declare_dram_parameter("wq", [DM, NH * HD], BF16, isOutput=False)
    wk = nc.declare_dram_parameter("wk", [DM, NH * HD], BF16, isOutput=False)
    wv = nc.declare_dram_parameter("wv", [DM, NH * HD], BF16, isOutput=False)
    wo = nc.declare_dram_parameter("wo", [NH * HD, DM], BF16, isOutput=False)
    cosR = nc.declare_dram_parameter("cosR", [S, NH * HD], BF16, isOutput=False)
    sinR = nc.declare_dram_parameter("sinR", [S, NH * HD], BF16, isOutput=False)
    masks = nc.declare_dram_parameter("masks", [128, 4 * 512], BF16, isOutput=False)
    y = nc.declare_dram_parameter("y", [S, DM], F32, isOutput=True)

    with tile.TileContext(nc) as tc:
        with tc.tile_pool(name="glob", bufs=1) as glob, \
             tc.tile_pool(name="p1w", bufs=1) as p1w, \
             tc.tile_pool(name="p1x", bufs=2) as p1x, \
             tc.tile_pool(name="p1s", bufs=3) as p1s, \
             tc.tile_pool(name="p1m", bufs=2) as p1m, \
             tc.tile_pool(name="qow", bufs=2) as qow, \
             tc.tile_pool(name="pt", bufs=4) as ptp, \
             tc.tile_pool(name="p2m", bufs=2) as p2m, \
             tc.tile_pool(name="p3w", bufs=1) as p3w, \
             tc.tile_pool(name="psP", bufs=1, space="PSUM") as psP, \
             tc.tile_pool(name="psS", bufs=2, space="PSUM") as psSp, \
             tc.tile_pool(name="psO", bufs=1, space="PSUM") as psOp, \
             tc.tile_pool(name="psD", bufs=1, space="PSUM") as psDp, \
             tc.tile_pool(name="psY", bufs=1, space="PSUM") as psYp:
            kT = glob.tile([128, NH, S], BF16, tag="kT", name="kT")
            vsb = [glob.tile([128, S], BF16, tag=f"v{h}", name=f"v{h}")
                   for h in range(NH)]
            mask_sb = glob.tile([128, 4 * 512], BF16, tag="masks", name="masks")
            ones_col = glob.tile([128, 1], BF16, tag="ones", name="ones")
            nc.vector.memset(ones_col, 1.0)
            eps_t = glob.tile([128, 1], F32, tag="eps", name="eps")
            nc.vector.memset(eps_t, EPS)
            ones_row = glob.tile([1, 128], F32, tag="onesr", name="onesr")
            nc.vector.memset(ones_row, 1.0)
            nc.sync.dma_start(out=mask_sb, in_=masks[:])
            wo_sb = [p3w.tile([128, DM], BF16, tag=f"wo{h}", name=f"wo{h}")
                     for h in range(NH)]
            wq_sb = [p1w.tile([128, 512], BF16, tag=f"wq{d}", name=f"wq{d}")
                     for d in range(NDM)]
            wk_sb = [p1w.tile([128, 512], BF16, tag=f"wk{d}", name=f"wk{d}")
                     for d in range(NDM)]
            wv_sb = [p1w.tile([128, 512], BF16, tag=f"wv{d}", name=f"wv{d}")
                     for d in range(NDM)]

            def emit_proj_wave(rep, wave):
                xt_sb = [None] * NDM
                for d in range(NDM):
                    xt_sb[d] = p1x.tile([128, 512], BF16, tag=f"xt{d}",
                                        name=f"xt{d}")
                    nc.sync.dma_start(
                        out=xt_sb[d],
                        in_=xT[d * 128:(d + 1) * 128,
                               wave * 512:(wave + 1) * 512])
                    if rep == 0 and wave == 0:
                        nc.sync.dma_start(
                            out=wq_sb[d], in_=wq[d * 128:(d + 1) * 128, :])
                        nc.sync.dma_start(
                            out=wk_sb[d], in_=wk[d * 128:(d + 1) * 128, :])
                        nc.sync.dma_start(
                            out=wv_sb[d], in_=wv[d * 128:(d + 1) * 128, :])
                if rep == 0 and wave == 0:
                    for h in range(NH):
                        nc.sync.dma_start(out=wo_sb[h],
                                          in_=wo[h * 128:(h + 1) * 128, :])

                qTw = qow.tile([128, NH, 512], BF16, tag="qTw", name="qTw")
                yield qTw
                for tcl in range(4):
                    tcg = wave * 4 + tcl
                    psq = psP.tile([128, 512], F32, tag="psq", name="psq")
                    psk = psP.tile([128, 512], F32, tag="psk", name="psk")
                    psv = psP.tile([128, 512], F32, tag="psv", name="psv")
                    rawq = p1m.tile([128, 512], BF16, tag="rawq", name="rawq")
                    rawk = p1m.tile([128, 512], BF16, tag="rawk", name="rawk")
                    rawv = p1m.tile([128, 512], BF16, tag="rawv", name="rawv")
                    # complete each accumulation before starting the next so
                    # its ACT evacuation overlaps the remaining matmuls
                    for ps, w_sb, raw in ((psq, wq_sb, rawq), (psk, wk_sb, rawk),
                                          (psv, wv_sb, rawv)):
                        for d in range(NDM):
                            lhs = xt_sb[d][:, tcl * 128:(tcl + 1) * 128]
                            nc.tensor.matmul(ps, lhs, w_sb[d],
                                             start=(d == 0), stop=(d == NDM - 1))
                        nc.scalar.copy(raw, ps)
                    ssq = p1m.tile([128, 12], F32, tag="ssq", name="ssq")
                    sqscr = p1m.tile([128, 128], BF16, tag="sqscr", name="sqscr")
                    for i, raw in enumerate((rawq, rawk, rawv)):
                        for h in range(NH):
                            nc.scalar.activation(
                                sqscr, raw[:, h * 128:(h + 1) * 128], AF.Square,
                                accum_out=ssq[:, 4 * i + h:4 * i + h + 1])
                    s1 = p1m.tile([128, 12], F32, tag="s1", name="s1")
                    nc.scalar.activation(s1, ssq, AF.Sqrt,
                                         bias=eps_t[:], scale=1.0 / HD)
                    rs = p1m.tile([128, 12], F32, tag="rs", name="rs")
                    nc.vector.reciprocal(rs, s1)
                    for h in range(NH):
                        nc.vector.tensor_scalar_mul(
                            vsb[h][:, tcg * 128:(tcg + 1) * 128],
                            rawv[:, h * 128:(h + 1) * 128],
                            rs[:, 8 + h:9 + h])
                    cos_t = p1s.tile([128, NH, 128], BF16, tag="cos", name="cos")
                    sin_t = p1s.tile([128, NH, 128], BF16, tag="sin", name="sin")
                    nc.sync.dma_start(
                        out=cos_t,
                        in_=cosR[tcg * 128:(tcg + 1) * 128, :]
                        .rearrange("p (h d) -> p h d", h=NH))
                    nc.sync.dma_start(
                        out=sin_t,
                        in_=sinR[tcg * 128:(tcg + 1) * 128, :]
                        .rearrange("p (h d) -> p h d", h=NH))
                    for qk, raw in ((0, rawq), (1, rawk)):
                        qn = p1m.tile([128, NH, 128], BF16, tag=f"qn{qk}",
                                      name=f"qn{qk}")
                        for h in range(NH):
                            nc.vector.tensor_scalar_mul(
                                qn[:, h, :], raw[:, h * 128:(h + 1) * 128],
                                rs[:, 4 * qk + h:4 * qk + h + 1])
                        ct = p1m.tile([128, NH, 128], BF16, tag=f"ct{qk}",
                                      name=f"ct{qk}")
                        nc.vector.tensor_tensor(ct[:], qn[:], cos_t[:], OP.mult)
                        tt = p1m.tile([128, NH, 128], BF16, tag=f"tt{qk}",
                                      name=f"tt{qk}")
                        nc.vector.tensor_tensor(
                            tt[:, :, 0:64], qn[:, :, 64:128],
                            sin_t[:, :, 0:64], OP.mult)
                        nc.vector.tensor_tensor(
                            tt[:, :, 64:128], qn[:, :, 0:64],
                            sin_t[:, :, 64:128], OP.mult)
                        qr = p1m.tile([128, NH, 128], BF16, tag=f"qr{qk}",
                                      name=f"qr{qk}")
                        nc.vector.tensor_tensor(qr[:], ct[:], tt[:], OP.add)
                        # one call transposes all 4 heads:
                        # out[p, h, c] = in[c, h*128 + p]
                        if qk == 0:
                            dst = qTw[:, :, tcl * 128:(tcl + 1) * 128]
                        else:
                            dst = kT[:, :, tcg * 128:(tcg + 1) * 128]
                        nc.sync.dma_start_transpose(dst, qr[:, :, :])
                    yield tcl

            def emit_attn_oproj(wave, qTw):
                nkt = 4 * (wave + 1)
                oTw = [qow.tile([128, 512], BF16, tag=f"oTw{h}",
                                name=f"oTw{h}") for h in range(NH)]
                for h in range(NH):
                    psO = psOp.tile([128, 512], F32, tag="psO", name="psO")
                    psD = psDp.tile([1, 512], F32, tag="psD", name="psD")
                    for kt in range(nkt):
                        psS = psSp.tile([128, 512], F32, tag="psS", name="psS")
                        nc.tensor.matmul(
                            psS, kT[:, h, kt * 128:(kt + 1) * 128],
                            qTw[:, h, :], start=True, stop=True)
                        PT = ptp.tile([128, 512], BF16, tag="PT", name="PT")
                        nc.scalar.activation(PT, psS, AF.Exp, scale=SCALE)
                        j = kt - 4 * wave
                        if j >= 0:
                            nc.vector.tensor_tensor(
                                PT, PT, mask_sb[:, j * 512:(j + 1) * 512],
                                OP.mult)
                        nc.tensor.matmul(psD, ones_col, PT,
                                         start=(kt == 0), stop=(kt == nkt - 1))
                        nc.tensor.matmul(
                            psO, vsb[h][:, kt * 128:(kt + 1) * 128], PT,
                            start=(kt == 0), stop=(kt == nkt - 1))
                    rd = p2m.tile([1, 512], F32, tag="rd", name="rd")
                    nc.vector.reciprocal(rd, psD)
                    psB = psSp.tile([128, 512], F32, tag="psS", name="psB")
                    nc.tensor.matmul(psB, ones_row, rd, start=True, stop=True)
                    rdB = p2m.tile([128, 512], F32, tag="rdB", name="rdB")
                    nc.scalar.copy(rdB, psB)
                    nc.vector.tensor_tensor(oTw[h][:], psO, rdB, OP.mult)
                    yield h

                for tcl in range(4):
                    for dmc in range(4):
                        psY = psYp.tile([128, 512], F32, tag="psY", name="psY")
                        for h in range(NH):
                            nc.tensor.matmul(
                                psY,
                                oTw[h][:, tcl * 128:(tcl + 1) * 128],
                                wo_sb[h][:, dmc * 512:(dmc + 1) * 512],
                                start=(h == 0), stop=(h == NH - 1))
                        ysb = p2m.tile([128, 512], F32, tag="ysb",
                                       name="ysb", bufs=3)
                        nc.vector.tensor_copy(ysb, psY)
                        nc.sync.dma_start(
                            out=y[(wave * 4 + tcl) * 128:
                                  (wave * 4 + tcl + 1) * 128,
                                  dmc * 512:(dmc + 1) * 512],
                            in_=ysb)
                yield "oproj"

            for rep in range(repeat):
                pending = None
                for wave in range(4):
                    gen_attn = emit_attn_oproj(*pending) if pending else None
                    if gen_attn is not None:
                        next(gen_attn, None)           # attn head 0 (wave-1)
                    gen_proj = emit_proj_wave(rep, wave)
                    qTw = next(gen_proj)               # emits the loads
                    for tcl in range(4):
                        next(gen_proj, None)           # proj chunk tcl
                        if gen_attn is not None and tcl < 3:
                            next(gen_attn, None)       # attn head tcl+1 (wave-1)
                    if gen_attn is not None:
                        next(gen_attn, None)           # o_proj (wave-1)
                        for _ in gen_attn:
                            pass
                    pending = (wave, qTw)
                gen_attn = emit_attn_oproj(*pending)
                for _ in gen_attn:
                    pass

    return nc


_NC_CACHE = None
_NC_LEGALIZED = False


def get_nc(legalized=False):
    global _NC_CACHE, _NC_LEGALIZED
    if _NC_CACHE is None:
        _NC_CACHE = build_nc()
    if legalized and not _NC_LEGALIZED:
        legalize_multi_waits(_NC_CACHE)
        _NC_LEGALIZED = True
    return _NC_CACHE


def prep_core_inputs(hidden_states, cos, sin, w_qkv, w_o):
    """Build the 8 per-core input maps (host-side shard + layout transforms)."""
    bf = ml_dtypes.bfloat16
    f32 = np.float32
    hidden_states = np.asarray(hidden_states, dtype=f32)
    cos = np.asarray(cos, dtype=f32)
    sin = np.asarray(sin, dtype=f32)
    w_qkv = np.asarray(w_qkv, dtype=f32)
    w_o = np.asarray(w_o, dtype=f32)

    # rope tables, replicated per head, sign-folded sin
    sinm = np.concatenate([-sin[:, :64], sin[:, 64:]], axis=1)
    cosR = np.ascontiguousarray(np.tile(cos, (1, NH))).astype(bf)
    sinR = np.ascontiguousarray(np.tile(sinm, (1, NH))).astype(bf)

    # causal masks for the 4 diagonal offsets
    p = np.arange(128)[:, None]
    c = np.arange(512)[None, :]
    masks = np.concatenate(
        [(p + 128 * j <= c) for j in range(4)], axis=1).astype(bf)

    in_maps = []
    for core in range(8):
        b, hg = core // 4, core % 4
        r0 = hg * NH * HD
        xTc = np.ascontiguousarray(hidden_states[b].T).astype(bf)
        wqc = np.ascontiguousarray(w_qkv[r0:r0 + NH * HD, :].T).astype(bf)
        wkc = np.ascontiguousarray(
            w_qkv[H * HD + r0:H * HD + r0 + NH * HD, :].T).astype(bf)
        wvc = np.ascontiguousarray(
            w_qkv[2 * H * HD + r0:2 * H * HD + r0 + NH * HD, :].T).astype(bf)
        woc = np.ascontiguousarray(w_o[:, r0:r0 + NH * HD].T).astype(bf)
        in_maps.append({
            "xT": xTc, "wq": wqc, "wk": wkc, "wv": wvc, "wo": woc,
            "cosR": cosR, "sinR": sinR, "masks": masks,
        })
    return in_maps


def kernel(hidden_states, cos, sin, w_qkv, w_o):
    nc = get_nc(legalized=True)
    in_maps = prep_core_inputs(hidden_states, cos, sin, w_qkv, w_o)
    res = run_bass_kernel_spmd(nc, in_maps, core_ids=list(range(8)))
    parts = [r["y"] for r in res.results]
    out = np.stack([
        parts[0] + parts[1] + parts[2] + parts[3],
        parts[4] + parts[5] + parts[6] + parts[7],
    ]).astype(np.float32)
    return out
